# revision 22
# baseline (speedup 1.0000x reference)
"""Gumbel-softmax sample + symmetric scatter kernel for 8 trn2 NeuronCores.

Math: out[e] = sigmoid(((g0 - g1) + (gum0 - gum1)) / TEMP) with
gum_k = -log(-log(u_k + EPS) + EPS).  The scatter target is fully
deterministic: part 1 (first S*DEL_NUM elements) is a dense [S, DEL_NUM]
block at matrix[0:S, S:SZ]; part 2 is the strict upper triangle of the
bottom-right [DEL_NUM, DEL_NUM] block.  Output = matrix + matrix.T.

Device: each core computes a contiguous 1/8 of the E sigmoid values
(memory-bound elementwise map).  Host places the values into the
symmetric [SZ, SZ] output.

v4 design — fp8(e3m4) inputs, ACT-bound pipeline (per core, per chunk):
  DMA:  u, g streamed as float8_e3m4 (1 byte/value: halves input HBM
        traffic vs v3's f16; engine budget per pass per core ~
        ACT 40.6us > DMA 33.2 > DVE 30 > GPSIMD 29).
  ACT:  w' = Ln(u - DELTA)       fp8 in -> f16 out
        DELTA>0 folds BOTH the old negate+clamp DVE op and the
        u->1.0-collapse tail estimator into the activation bias:
        for collapsed u==1.0, w' = ln(DELTA) = -E[gum | u rounds to 1].
  DVE:  l = C1*bits_i16(w') + C0S   fused second-log bit trick; the
        i16 view of the NEGATIVE w' equals bits(|w'|) - 32768, so the
        sign flip is absorbed into the constant C0S = C0 + 32768*C1.
  DVE:  dg = g0 - g1 (fp8 tensor_tensor), dl = l1 - l0
  GPSIMD: s = dg + dl            (keeps DVE under the ACT roofline)
then sigmoids batched last (single Ln->Sigmoid table switch per pass):
  ACT:  out = Sigmoid(s / TEMP), DMA out (f16).

Numerics: host-side e3m4 conversion is pure dtype quantization (round-
to-nearest, u floored at 2^-6 so Ln(u-DELTA) is always finite).  The
dominant error is the e3m4 u-quantization near u=1 (top band collapses
to code 1.0, decoded as the band's conditional-mean gumbel via DELTA);
measured end-to-end rel_fro vs f64 reference ~1.1e-2 (gate: 2e-2).

DMA order: u chunks run two ahead of g chunks so the ACT Ln pass (paced
only by u arrivals) never starves; sigmoid batch overlaps the tail of
the g-load/DVE/GPSIMD pipeline.
"""

import math
import os

import numpy as np

SZ = 8192
DEL_NUM = 2048
S = SZ - DEL_NUM               # 6144
E1 = S * DEL_NUM               # 12,582,912 dense block elements
E2 = DEL_NUM * (DEL_NUM - 1) // 2  # 2,096,128 triangular elements
E = E1 + E2                    # 14,679,040
NCORES = 8
CH = E // NCORES               # 1,834,880 elements per core
P = 128
FTOT = CH // P                 # 14,335 outputs per partition
# Supertile widths: ACT instructions are issued once per supertile.  HW
# measures ~1us fixed overhead per ACT instruction (vs ~0.4us modeled),
# so few/large ACT instructions win; DMA stays finer-grained (per
# component per supertile) for overlap.  First supertile small for fill.
SIZES = [2048, 4096, 4096, 4095]
OFFS = [sum(SIZES[:i]) for i in range(len(SIZES))]
NCHUNK = len(SIZES)
assert sum(SIZES) == FTOT
# Sigmoid batch split points (supertile-aligned, 2 ACT instructions).
SIG_SPLITS = [(0, 6144), (6144, FTOT)]
# st = dg + dl engine assignment: GPSIMD for the early chunks (keeps DVE
# under the ACT roofline), DVE for the tail (GPSIMD's ~2ns/elem latency
# would cascade into the sigmoid drain).  Env knobs are A/B-test hooks;
# the defaults are the production config.
U_DTYPE = os.environ.get("K_U_DTYPE", "f8e3")    # f8e3 | f8e4 | f16
G_DTYPE = os.environ.get("K_G_DTYPE", "f8e3")    # f8e3 | f8e4 | f16
DELTA16 = math.exp(-9.32)      # f16 u->1.0 collapse: E[gum | u>1-2^-12]
DELTA8E4 = math.exp(-4.45)     # e4m3 u->1.0 collapse: E[gum | u>0.969]
U8E4_FLOOR = 0.0234375         # e4m3 floor: > 2*DELTA8E4 so Ln arg > 0
TEMP = 10.0
LN2 = math.log(2.0)
C1_16 = LN2 / 1024.0           # f16 bits: i = 1024*(e_biased + m)
C0S_16 = LN2 * (32768.0 / 1024.0 - 15.0 + 0.0430357)
DELTA8 = math.exp(-5.15)       # e3m4 u->1.0 collapse: E[gum | u>0.984]
U8_FLOOR = 2.0 ** -6           # smallest nonzero e3m4 code (see staging)

_cache = {}


def _build(nrep=None):
    """Build the SPMD program.  nrep=None -> production single pass;
    nrep=N wraps the identical pass in a device-side For_i loop (timing
    instrument: one NEFF execution runs the pass N times back-to-back)."""
    import concourse.bacc as bacc
    import concourse.mybir as mybir
    import concourse.tile as tile

    f16 = mybir.dt.float16
    f32 = mybir.dt.float32
    f8e3 = mybir.dt.float8e3
    i16 = mybir.dt.int16
    AF = mybir.ActivationFunctionType

    nc = bacc.Bacc(
        "TRN2", target_bir_lowering=False, debug=False, num_devices=NCORES
    )

    f8e4 = mybir.dt.float8e4
    dt_map = {"f8e3": f8e3, "f8e4": f8e4, "f16": f16}
    u_dt = dt_map[U_DTYPE]
    g_dt = dt_map[G_DTYPE]
    delta = {"f8e3": DELTA8, "f8e4": DELTA8E4, "f16": DELTA16}[U_DTYPE]

    # Float activation biases require registered const APs.
    for val in (-delta,):
        t = nc.alloc_sbuf_tensor(f"const-f32-{val}", [128, 1], f32)
        nc.gpsimd.memset(t.ap(), val)
        nc.const_aps.aps[(f32, val)] = t.ap()
    nc.all_engine_barrier()

    u_ap = nc.dram_tensor("u", [P, 2 * FTOT], u_dt, kind="ExternalInput").ap()
    g_ap = nc.dram_tensor("gen", [P, 2 * FTOT], g_dt, kind="ExternalInput").ap()
    out_ap = nc.dram_tensor("out", [P, FTOT], f16, kind="ExternalOutput").ap()

    with tile.TileContext(nc) as tc:
        with tc.tile_pool(name="pool", bufs=2) as pool:

            def one_pass():
                uts = {}

                def load_u(i):
                    Fi, Oi = SIZES[i], OFFS[i]
                    uts[i] = pool.tile(
                        [P, 2 * Fi], u_dt, tag="u", bufs=3, name=f"ut{i}"
                    )
                    # Two DMAs per supertile (one per component half) so
                    # transfers overlap compute at sub-supertile grain.
                    nc.sync.dma_start(
                        uts[i][:, 0:Fi], u_ap[:, 2 * Oi : 2 * Oi + Fi]
                    )
                    nc.sync.dma_start(
                        uts[i][:, Fi : 2 * Fi],
                        u_ap[:, 2 * Oi + Fi : 2 * (Oi + Fi)],
                    )

                load_u(0)
                load_u(1)
                st = pool.tile([P, FTOT], f16, tag="s", bufs=1, name="st")
                for i, (Fi, Oi) in enumerate(zip(SIZES, OFFS)):
                    if i + 2 < NCHUNK:
                        load_u(i + 2)
                    gt = pool.tile([P, 2 * Fi], g_dt, tag="g", bufs=3, name=f"gt{i}")
                    nc.sync.dma_start(
                        gt[:, 0:Fi], g_ap[:, 2 * Oi : 2 * Oi + Fi]
                    )
                    nc.sync.dma_start(
                        gt[:, Fi : 2 * Fi], g_ap[:, 2 * Oi + Fi : 2 * (Oi + Fi)]
                    )
                    ut = uts.pop(i)

                    wt = pool.tile([P, 2 * Fi], f16, tag="w", bufs=2)
                    nc.scalar.activation(wt[:], ut[:], AF.Ln, bias=-delta)
                    lt = pool.tile([P, 2 * Fi], f16, tag="l", bufs=2)
                    nc.vector.tensor_scalar(
                        lt[:], wt[:].bitcast(i16), C1_16, C0S_16,
                        op0=mybir.AluOpType.mult, op1=mybir.AluOpType.add,
                    )

                    dgt = pool.tile([P, Fi], f16, tag="dg", bufs=2)
                    nc.vector.tensor_sub(dgt[:], gt[:, 0:Fi], gt[:, Fi : 2 * Fi])
                    dlt = pool.tile([P, Fi], f16, tag="dl", bufs=2)
                    nc.vector.tensor_sub(dlt[:], lt[:, Fi : 2 * Fi], lt[:, 0:Fi])
                    nc.vector.tensor_add(st[:, Oi : Oi + Fi], dgt[:], dlt[:])

                # Sigmoids batched into 2 big instructions (one Ln->Sigmoid
                # table switch per pass); out-DMA per supertile range.
                for lo, hi in SIG_SPLITS:
                    nc.scalar.activation(
                        st[:, lo:hi], st[:, lo:hi], AF.Sigmoid, scale=1.0 / TEMP
                    )
                for Fi, Oi in zip(SIZES, OFFS):
                    nc.sync.dma_start(out_ap[:, Oi : Oi + Fi], st[:, Oi : Oi + Fi])

            if nrep is None:
                one_pass()
            elif isinstance(nrep, tuple) and nrep[0] == "unroll":
                for _ in range(nrep[1]):
                    one_pass()
            else:
                with tc.For_i(0, nrep):
                    one_pass()

    nc.compile()
    return nc


def get_nc(nrep=None):
    if nrep not in _cache:
        _cache[nrep] = _build(nrep)
    return _cache[nrep]


def _conv_u(a32):
    """u -> device dtype, floored at the smallest nonzero code so the
    device-side Ln(u - delta) is always finite."""
    import ml_dtypes

    if U_DTYPE == "f8e3":
        q = a32.astype(ml_dtypes.float8_e3m4)
        return np.maximum(q, ml_dtypes.float8_e3m4(U8_FLOOR))
    if U_DTYPE == "f8e4":
        q = a32.astype(ml_dtypes.float8_e4m3)
        return np.maximum(q, ml_dtypes.float8_e4m3(U8E4_FLOOR))
    q = a32.astype(np.float16)
    return np.maximum(q, np.float16(2.0 ** -12))


def _conv_g(a32):
    import ml_dtypes

    if G_DTYPE == "f8e3":
        return a32.astype(ml_dtypes.float8_e3m4)
    if G_DTYPE == "f8e4":
        return a32.astype(ml_dtypes.float8_e4m3)
    return a32.astype(np.float16)


def stage_core_inputs(arr: np.ndarray, core: int, kind: str) -> np.ndarray:
    """Slice one core's [CH, 2] block and lay it out as [P, 2*FTOT]:
    within chunk i, component-0 values occupy the first Fi columns and
    component-1 the next Fi (unit-stride halves for the engines)."""
    conv = _conv_u if kind == "u" else _conv_g
    a = conv(arr[core * CH : (core + 1) * CH])
    a = a.reshape(P, FTOT, 2)
    out = np.empty((P, 2 * FTOT), a.dtype)
    for Fi, Oi in zip(SIZES, OFFS):
        blk = a[:, Oi : Oi + Fi, :]
        out[:, 2 * Oi : 2 * Oi + Fi] = blk[:, :, 0]
        out[:, 2 * Oi + Fi : 2 * (Oi + Fi)] = blk[:, :, 1]
    return out


def core_input_map(gen: np.ndarray, u: np.ndarray, core: int) -> dict:
    return {
        "gen": stage_core_inputs(gen, core, "g"),
        "u": stage_core_inputs(u, core, "u"),
    }


def run_cores(gen: np.ndarray, u: np.ndarray, trace: bool = False):
    """Run the SPMD kernel on flat [E, 2] inputs; returns (flat out [E], results obj)."""
    from concourse.bass_utils import run_bass_kernel_spmd

    nc = get_nc()
    in_maps = [core_input_map(gen, u, c) for c in range(NCORES)]
    kw = {}
    if trace:
        kw = {"trace": True, "trace_cores": list(range(NCORES)), "stitch_traces": True}
    res = run_bass_kernel_spmd(nc, in_maps, core_ids=list(range(NCORES)), **kw)
    out = np.concatenate(
        [np.asarray(r["out"]).astype(np.float32).reshape(-1) for r in res.results]
    )
    return out, res


def assemble(out: np.ndarray) -> np.ndarray:
    full = np.zeros((SZ, SZ), np.float32)
    a = out[:E1].reshape(S, DEL_NUM)
    full[:S, S:] = a
    full[S:, :S] = a.T
    ti, tj = np.triu_indices(DEL_NUM, k=1)
    b = np.zeros((DEL_NUM, DEL_NUM), np.float32)
    b[ti, tj] = out[E1:]
    full[S:, S:] = b + b.T
    return full


def kernel(gen_matrix=None, u=None, sz=None, del_num=None, **_ignored):
    gen = np.ascontiguousarray(np.asarray(gen_matrix, dtype=np.float32))
    uu = np.ascontiguousarray(np.asarray(u, dtype=np.float32))
    assert gen.shape == (E, 2) and uu.shape == (E, 2)
    out, _ = run_cores(gen, uu)
    return assemble(out)


# revision 25
# speedup vs baseline: 1.0588x; 1.0588x over previous
"""Gumbel-softmax sample + symmetric scatter kernel for 8 trn2 NeuronCores.

Math: out[e] = sigmoid(((g0 - g1) + (gum0 - gum1)) / TEMP) with
gum_k = -log(-log(u_k + EPS) + EPS).  The scatter target is fully
deterministic: part 1 (first S*DEL_NUM elements) is a dense [S, DEL_NUM]
block at matrix[0:S, S:SZ]; part 2 is the strict upper triangle of the
bottom-right [DEL_NUM, DEL_NUM] block.  Output = matrix + matrix.T.

Device: each core computes a contiguous 1/8 of the E sigmoid values
(memory-bound elementwise map).  Host places the values into the
symmetric [SZ, SZ] output.

v4 design — fp8(e3m4) inputs, ACT-bound pipeline (per core, per chunk):
  DMA:  u, g streamed as float8_e3m4 (1 byte/value: halves input HBM
        traffic vs v3's f16; engine budget per pass per core ~
        ACT 40.6us > DMA 33.2 > DVE 30 > GPSIMD 29).
  ACT:  w' = Ln(u - DELTA)       fp8 in -> f16 out
        DELTA>0 folds BOTH the old negate+clamp DVE op and the
        u->1.0-collapse tail estimator into the activation bias:
        for collapsed u==1.0, w' = ln(DELTA) = -E[gum | u rounds to 1].
  DVE:  l = C1*bits_i16(w') + C0S   fused second-log bit trick; the
        i16 view of the NEGATIVE w' equals bits(|w'|) - 32768, so the
        sign flip is absorbed into the constant C0S = C0 + 32768*C1.
  DVE:  dg = g0 - g1 (fp8 tensor_tensor), dl = l1 - l0
  GPSIMD: s = dg + dl            (keeps DVE under the ACT roofline)
then sigmoids batched last (single Ln->Sigmoid table switch per pass):
  ACT:  out = Sigmoid(s / TEMP), DMA out (f16).

Numerics: host-side e3m4 conversion is pure dtype quantization (round-
to-nearest, u floored at 2^-6 so Ln(u-DELTA) is always finite).  The
dominant error is the e3m4 u-quantization near u=1 (top band collapses
to code 1.0, decoded as the band's conditional-mean gumbel via DELTA);
measured end-to-end rel_fro vs f64 reference ~1.1e-2 (gate: 2e-2).

DMA order: u chunks run two ahead of g chunks so the ACT Ln pass (paced
only by u arrivals) never starves; sigmoid batch overlaps the tail of
the g-load/DVE/GPSIMD pipeline.
"""

import math
import os

import numpy as np

SZ = 8192
DEL_NUM = 2048
S = SZ - DEL_NUM               # 6144
E1 = S * DEL_NUM               # 12,582,912 dense block elements
E2 = DEL_NUM * (DEL_NUM - 1) // 2  # 2,096,128 triangular elements
E = E1 + E2                    # 14,679,040
NCORES = 8
CH = E // NCORES               # 1,834,880 elements per core
P = 128
FTOT = CH // P                 # 14,335 outputs per partition
# Supertile widths: ACT instructions are issued once per supertile.  HW
# measures ~1us fixed overhead per ACT instruction (vs ~0.4us modeled),
# so few/large ACT instructions win; the tail shrinks so the last
# supertile's DVE chain + sigmoid + out-DMA drain stays short.
SIZES = [4096, 4096, 3072, 2048, 1023]
OFFS = [sum(SIZES[:i]) for i in range(len(SIZES))]
NCHUNK = len(SIZES)
assert sum(SIZES) == FTOT
# Sigmoid batch split points (supertile-aligned ACT instructions).
SIG_SPLITS = [(0, 8192), (8192, 13312), (13312, FTOT)]
# st = dg + dl engine assignment: GPSIMD for the early chunks (keeps DVE
# under the ACT roofline), DVE for the tail (GPSIMD's ~2ns/elem latency
# would cascade into the sigmoid drain).  Env knobs are A/B-test hooks;
# the defaults are the production config.
U_DTYPE = os.environ.get("K_U_DTYPE", "f8e3")    # f8e3 | f8e4 | f16
G_DTYPE = os.environ.get("K_G_DTYPE", "f8e3")    # f8e3 | f8e4 | f16
DELTA16 = math.exp(-9.32)      # f16 u->1.0 collapse: E[gum | u>1-2^-12]
DELTA8E4 = math.exp(-4.45)     # e4m3 u->1.0 collapse: E[gum | u>0.969]
U8E4_FLOOR = 0.0234375         # e4m3 floor: > 2*DELTA8E4 so Ln arg > 0
TEMP = 10.0
LN2 = math.log(2.0)
C1_16 = LN2 / 1024.0           # f16 bits: i = 1024*(e_biased + m)
C0S_16 = LN2 * (32768.0 / 1024.0 - 15.0 + 0.0430357)
DELTA8 = math.exp(-5.15)       # e3m4 u->1.0 collapse: E[gum | u>0.984]
U8_FLOOR = 2.0 ** -6           # smallest nonzero e3m4 code (see staging)

_cache = {}


def _build(nrep=None):
    """Build the SPMD program.  nrep=None -> production single pass;
    nrep=N wraps the identical pass in a device-side For_i loop (timing
    instrument: one NEFF execution runs the pass N times back-to-back)."""
    import concourse.bacc as bacc
    import concourse.mybir as mybir
    import concourse.tile as tile

    f16 = mybir.dt.float16
    f32 = mybir.dt.float32
    f8e3 = mybir.dt.float8e3
    i16 = mybir.dt.int16
    AF = mybir.ActivationFunctionType

    nc = bacc.Bacc(
        "TRN2", target_bir_lowering=False, debug=False, num_devices=NCORES
    )

    f8e4 = mybir.dt.float8e4
    dt_map = {"f8e3": f8e3, "f8e4": f8e4, "f16": f16}
    u_dt = dt_map[U_DTYPE]
    g_dt = dt_map[G_DTYPE]
    delta = {"f8e3": DELTA8, "f8e4": DELTA8E4, "f16": DELTA16}[U_DTYPE]

    # Float activation biases require registered const APs.
    for val in (-delta,):
        t = nc.alloc_sbuf_tensor(f"const-f32-{val}", [128, 1], f32)
        nc.gpsimd.memset(t.ap(), val)
        nc.const_aps.aps[(f32, val)] = t.ap()
    nc.all_engine_barrier()

    u_ap = nc.dram_tensor("u", [P, 2 * FTOT], u_dt, kind="ExternalInput").ap()
    g_ap = nc.dram_tensor("gen", [P, 2 * FTOT], g_dt, kind="ExternalInput").ap()
    out_ap = nc.dram_tensor("out", [P, FTOT], f16, kind="ExternalOutput").ap()

    with tile.TileContext(nc) as tc:
        with tc.tile_pool(name="pool", bufs=2) as pool:

            def one_pass():
                uts = {}

                def load_u(i):
                    Fi, Oi = SIZES[i], OFFS[i]
                    uts[i] = pool.tile(
                        [P, 2 * Fi], u_dt, tag="u", bufs=3, name=f"ut{i}"
                    )
                    # Two DMAs per supertile (one per component half) so
                    # transfers overlap compute at sub-supertile grain.
                    nc.sync.dma_start(
                        uts[i][:, 0:Fi], u_ap[:, 2 * Oi : 2 * Oi + Fi]
                    )
                    nc.sync.dma_start(
                        uts[i][:, Fi : 2 * Fi],
                        u_ap[:, 2 * Oi + Fi : 2 * (Oi + Fi)],
                    )

                load_u(0)
                st = pool.tile([P, FTOT], f16, tag="s", bufs=2, name="st")
                sig_done = 0
                for i, (Fi, Oi) in enumerate(zip(SIZES, OFFS)):
                    if i + 1 < NCHUNK:
                        load_u(i + 1)
                    gt = pool.tile([P, 2 * Fi], g_dt, tag="g", bufs=2, name=f"gt{i}")
                    nc.sync.dma_start(
                        gt[:, 0:Fi], g_ap[:, 2 * Oi : 2 * Oi + Fi]
                    )
                    nc.sync.dma_start(
                        gt[:, Fi : 2 * Fi], g_ap[:, 2 * Oi + Fi : 2 * (Oi + Fi)]
                    )
                    ut = uts.pop(i)

                    # dg first in each DVE block: it only needs the g DMA, so
                    # the in-order DVE stream never stalls on it, and the
                    # post-Ln tail chain is just ts -> dl -> add.
                    dgt = pool.tile([P, Fi], f16, tag="dg", bufs=2)
                    nc.vector.tensor_sub(dgt[:], gt[:, 0:Fi], gt[:, Fi : 2 * Fi])

                    wt = pool.tile([P, 2 * Fi], f16, tag="w", bufs=2)
                    nc.scalar.activation(wt[:], ut[:], AF.Ln, bias=-delta)
                    lt = pool.tile([P, 2 * Fi], f16, tag="l", bufs=2)
                    nc.vector.tensor_scalar(
                        lt[:], wt[:].bitcast(i16), C1_16, C0S_16,
                        op0=mybir.AluOpType.mult, op1=mybir.AluOpType.add,
                    )
                    dlt = pool.tile([P, Fi], f16, tag="dl", bufs=2)
                    nc.vector.tensor_sub(dlt[:], lt[:, Fi : 2 * Fi], lt[:, 0:Fi])
                    nc.vector.tensor_add(st[:, Oi : Oi + Fi], dgt[:], dlt[:])

                # Sigmoids batched after all Lns (one table switch per pass),
                # split so the first batches run while the tail supertiles'
                # DVE chains finish; out-DMA right behind each batch.
                for lo, hi in SIG_SPLITS:
                    nc.scalar.activation(
                        st[:, lo:hi], st[:, lo:hi], AF.Sigmoid, scale=1.0 / TEMP
                    )
                    nc.sync.dma_start(out_ap[:, lo:hi], st[:, lo:hi])

            if nrep is None:
                one_pass()
            elif isinstance(nrep, tuple) and nrep[0] == "unroll":
                for _ in range(nrep[1]):
                    one_pass()
            else:
                with tc.For_i(0, nrep):
                    one_pass()

    nc.compile()
    return nc


def get_nc(nrep=None):
    if nrep not in _cache:
        _cache[nrep] = _build(nrep)
    return _cache[nrep]


def _conv_u(a32):
    """u -> device dtype, floored at the smallest nonzero code so the
    device-side Ln(u - delta) is always finite."""
    import ml_dtypes

    if U_DTYPE == "f8e3":
        q = a32.astype(ml_dtypes.float8_e3m4)
        return np.maximum(q, ml_dtypes.float8_e3m4(U8_FLOOR))
    if U_DTYPE == "f8e4":
        q = a32.astype(ml_dtypes.float8_e4m3)
        return np.maximum(q, ml_dtypes.float8_e4m3(U8E4_FLOOR))
    q = a32.astype(np.float16)
    return np.maximum(q, np.float16(2.0 ** -12))


def _conv_g(a32):
    import ml_dtypes

    if G_DTYPE == "f8e3":
        return a32.astype(ml_dtypes.float8_e3m4)
    if G_DTYPE == "f8e4":
        return a32.astype(ml_dtypes.float8_e4m3)
    return a32.astype(np.float16)


def stage_core_inputs(arr: np.ndarray, core: int, kind: str) -> np.ndarray:
    """Slice one core's [CH, 2] block and lay it out as [P, 2*FTOT]:
    within chunk i, component-0 values occupy the first Fi columns and
    component-1 the next Fi (unit-stride halves for the engines)."""
    conv = _conv_u if kind == "u" else _conv_g
    a = conv(arr[core * CH : (core + 1) * CH])
    a = a.reshape(P, FTOT, 2)
    out = np.empty((P, 2 * FTOT), a.dtype)
    for Fi, Oi in zip(SIZES, OFFS):
        blk = a[:, Oi : Oi + Fi, :]
        out[:, 2 * Oi : 2 * Oi + Fi] = blk[:, :, 0]
        out[:, 2 * Oi + Fi : 2 * (Oi + Fi)] = blk[:, :, 1]
    return out


def core_input_map(gen: np.ndarray, u: np.ndarray, core: int) -> dict:
    return {
        "gen": stage_core_inputs(gen, core, "g"),
        "u": stage_core_inputs(u, core, "u"),
    }


def run_cores(gen: np.ndarray, u: np.ndarray, trace: bool = False):
    """Run the SPMD kernel on flat [E, 2] inputs; returns (flat out [E], results obj)."""
    from concourse.bass_utils import run_bass_kernel_spmd

    nc = get_nc()
    in_maps = [core_input_map(gen, u, c) for c in range(NCORES)]
    kw = {}
    if trace:
        kw = {"trace": True, "trace_cores": list(range(NCORES)), "stitch_traces": True}
    res = run_bass_kernel_spmd(nc, in_maps, core_ids=list(range(NCORES)), **kw)
    out = np.concatenate(
        [np.asarray(r["out"]).astype(np.float32).reshape(-1) for r in res.results]
    )
    return out, res


def assemble(out: np.ndarray) -> np.ndarray:
    full = np.zeros((SZ, SZ), np.float32)
    a = out[:E1].reshape(S, DEL_NUM)
    full[:S, S:] = a
    full[S:, :S] = a.T
    ti, tj = np.triu_indices(DEL_NUM, k=1)
    b = np.zeros((DEL_NUM, DEL_NUM), np.float32)
    b[ti, tj] = out[E1:]
    full[S:, S:] = b + b.T
    return full


def kernel(gen_matrix=None, u=None, sz=None, del_num=None, **_ignored):
    gen = np.ascontiguousarray(np.asarray(gen_matrix, dtype=np.float32))
    uu = np.ascontiguousarray(np.asarray(u, dtype=np.float32))
    assert gen.shape == (E, 2) and uu.shape == (E, 2)
    out, _ = run_cores(gen, uu)
    return assemble(out)


# revision 29
# speedup vs baseline: 1.1095x; 1.0479x over previous
"""Gumbel-softmax sample + symmetric scatter kernel for 8 trn2 NeuronCores.

Math: out[e] = sigmoid(((g0 - g1) + (gum0 - gum1)) / TEMP) with
gum_k = -log(-log(u_k + EPS) + EPS).  The scatter target is fully
deterministic: part 1 (first S*DEL_NUM elements) is a dense [S, DEL_NUM]
block at matrix[0:S, S:SZ]; part 2 is the strict upper triangle of the
bottom-right [DEL_NUM, DEL_NUM] block.  Output = matrix + matrix.T.

Device: each core computes a contiguous 1/8 of the E sigmoid values
(memory-bound elementwise map).  Host places the values into the
symmetric [SZ, SZ] output.

v4 design — fp8(e3m4) inputs, ACT-bound pipeline (per core, per chunk):
  DMA:  u, g streamed as float8_e3m4 (1 byte/value: halves input HBM
        traffic vs v3's f16; engine budget per pass per core ~
        ACT 40.6us > DMA 33.2 > DVE 30 > GPSIMD 29).
  ACT:  w' = Ln(u - DELTA)       fp8 in -> f16 out
        DELTA>0 folds BOTH the old negate+clamp DVE op and the
        u->1.0-collapse tail estimator into the activation bias:
        for collapsed u==1.0, w' = ln(DELTA) = -E[gum | u rounds to 1].
  DVE:  l = C1*bits_i16(w') + C0S   fused second-log bit trick; the
        i16 view of the NEGATIVE w' equals bits(|w'|) - 32768, so the
        sign flip is absorbed into the constant C0S = C0 + 32768*C1.
  DVE:  dg = g0 - g1 (fp8 tensor_tensor), dl = l1 - l0
  GPSIMD: s = dg + dl            (keeps DVE under the ACT roofline)
then sigmoids batched last (single Ln->Sigmoid table switch per pass):
  ACT:  out = Sigmoid(s / TEMP), DMA out (f16).

Numerics: host-side e3m4 conversion is pure dtype quantization (round-
to-nearest, u floored at 2^-6 so Ln(u-DELTA) is always finite).  The
dominant error is the e3m4 u-quantization near u=1 (top band collapses
to code 1.0, decoded as the band's conditional-mean gumbel via DELTA);
measured end-to-end rel_fro vs f64 reference ~1.1e-2 (gate: 2e-2).

DMA order: u chunks run two ahead of g chunks so the ACT Ln pass (paced
only by u arrivals) never starves; sigmoid batch overlaps the tail of
the g-load/DVE/GPSIMD pipeline.
"""

import math
import os

import numpy as np

SZ = 8192
DEL_NUM = 2048
S = SZ - DEL_NUM               # 6144
E1 = S * DEL_NUM               # 12,582,912 dense block elements
E2 = DEL_NUM * (DEL_NUM - 1) // 2  # 2,096,128 triangular elements
E = E1 + E2                    # 14,679,040
NCORES = 8
CH = E // NCORES               # 1,834,880 elements per core
P = 128
FTOT = CH // P                 # 14,335 outputs per partition
# Supertile widths: every engine instruction covers one whole supertile
# (no subtile writes anywhere -- the tile framework's subtile dependency
# tracking serializes badly on HW).  ACT instructions carry ~1us fixed
# overhead each on HW, so few/large instructions win; the last supertile
# is half-size so its DVE chain + sigmoid hide under the earlier
# sigmoids instead of draining after them.
SIZES = [4096, 4096, 4096, 2047]
OFFS = [sum(SIZES[:i]) for i in range(len(SIZES))]
NCHUNK = len(SIZES)
assert sum(SIZES) == FTOT
# st = dg + dl engine assignment: GPSIMD for the early chunks (keeps DVE
# under the ACT roofline), DVE for the tail (GPSIMD's ~2ns/elem latency
# would cascade into the sigmoid drain).  Env knobs are A/B-test hooks;
# the defaults are the production config.
U_DTYPE = os.environ.get("K_U_DTYPE", "f8e3")    # f8e3 | f8e4 | f16
G_DTYPE = os.environ.get("K_G_DTYPE", "f8e3")    # f8e3 | f8e4 | f16
DELTA16 = math.exp(-9.32)      # f16 u->1.0 collapse: E[gum | u>1-2^-12]
DELTA8E4 = math.exp(-4.45)     # e4m3 u->1.0 collapse: E[gum | u>0.969]
U8E4_FLOOR = 0.0234375         # e4m3 floor: > 2*DELTA8E4 so Ln arg > 0
TEMP = 10.0
LN2 = math.log(2.0)
C1_16 = LN2 / 1024.0           # f16 bits: i = 1024*(e_biased + m)
C0S_16 = LN2 * (32768.0 / 1024.0 - 15.0 + 0.0430357)
DELTA8 = math.exp(-5.15)       # e3m4 u->1.0 collapse: E[gum | u>0.984]
U8_FLOOR = 2.0 ** -6           # smallest nonzero e3m4 code (see staging)

_cache = {}


def _build(nrep=None):
    """Build the SPMD program.  nrep=None -> production single pass;
    nrep=N wraps the identical pass in a device-side For_i loop (timing
    instrument: one NEFF execution runs the pass N times back-to-back)."""
    import concourse.bacc as bacc
    import concourse.mybir as mybir
    import concourse.tile as tile

    f16 = mybir.dt.float16
    f32 = mybir.dt.float32
    f8e3 = mybir.dt.float8e3
    i16 = mybir.dt.int16
    AF = mybir.ActivationFunctionType

    nc = bacc.Bacc(
        "TRN2", target_bir_lowering=False, debug=False, num_devices=NCORES
    )

    f8e4 = mybir.dt.float8e4
    dt_map = {"f8e3": f8e3, "f8e4": f8e4, "f16": f16}
    u_dt = dt_map[U_DTYPE]
    g_dt = dt_map[G_DTYPE]
    delta = {"f8e3": DELTA8, "f8e4": DELTA8E4, "f16": DELTA16}[U_DTYPE]

    # Float activation biases require registered const APs.
    for val in (-delta,):
        t = nc.alloc_sbuf_tensor(f"const-f32-{val}", [128, 1], f32)
        nc.gpsimd.memset(t.ap(), val)
        nc.const_aps.aps[(f32, val)] = t.ap()
    nc.all_engine_barrier()

    u_ap = nc.dram_tensor("u", [P, 2 * FTOT], u_dt, kind="ExternalInput").ap()
    g_ap = nc.dram_tensor("gen", [P, 2 * FTOT], g_dt, kind="ExternalInput").ap()
    out_ap = nc.dram_tensor("out", [P, FTOT], f16, kind="ExternalOutput").ap()

    with tile.TileContext(nc) as tc:
        with tc.tile_pool(name="pool", bufs=2) as pool:

            def one_pass():
                uts = {}

                def load_u(i):
                    Fi, Oi = SIZES[i], OFFS[i]
                    uts[i] = pool.tile(
                        [P, 2 * Fi], u_dt, tag="u", bufs=2, name=f"ut{i}"
                    )
                    nc.sync.dma_start(uts[i][:], u_ap[:, 2 * Oi : 2 * (Oi + Fi)])

                load_u(0)
                st_list = []
                for i, (Fi, Oi) in enumerate(zip(SIZES, OFFS)):
                    if i + 1 < NCHUNK:
                        load_u(i + 1)
                    gt = pool.tile([P, 2 * Fi], g_dt, tag="g", bufs=2, name=f"gt{i}")
                    nc.sync.dma_start(gt[:], g_ap[:, 2 * Oi : 2 * (Oi + Fi)])
                    ut = uts.pop(i)

                    # dg first in each DVE block: it only needs the g DMA, so
                    # the in-order DVE stream never stalls on it, and the
                    # post-Ln tail chain is just ts -> dl -> add.
                    dgt = pool.tile([P, Fi], f16, tag="dg", bufs=2)
                    nc.vector.tensor_sub(dgt[:], gt[:, 0:Fi], gt[:, Fi : 2 * Fi])

                    wt = pool.tile([P, 2 * Fi], f16, tag="w", bufs=2)
                    nc.scalar.activation(wt[:], ut[:], AF.Ln, bias=-delta)
                    lt = pool.tile([P, 2 * Fi], f16, tag="l", bufs=2)
                    nc.vector.tensor_scalar(
                        lt[:], wt[:].bitcast(i16), C1_16, C0S_16,
                        op0=mybir.AluOpType.mult, op1=mybir.AluOpType.add,
                    )
                    dlt = pool.tile([P, Fi], f16, tag="dl", bufs=2)
                    nc.vector.tensor_sub(dlt[:], lt[:, Fi : 2 * Fi], lt[:, 0:Fi])
                    st = pool.tile([P, Fi], f16, tag="s", bufs=NCHUNK + 1,
                                   name=f"st{i}")
                    nc.vector.tensor_add(st[:], dgt[:], dlt[:])
                    st_list.append((st, Oi, Fi))

                # Sigmoids batched after all Lns (one table switch per pass);
                # out-DMA right behind each one.
                for st, Oi, Fi in st_list:
                    nc.scalar.activation(st[:], st[:], AF.Sigmoid, scale=1.0 / TEMP)
                    nc.sync.dma_start(out_ap[:, Oi : Oi + Fi], st[:])

            if nrep is None:
                one_pass()
            elif isinstance(nrep, tuple) and nrep[0] == "unroll":
                for _ in range(nrep[1]):
                    one_pass()
            else:
                with tc.For_i(0, nrep):
                    one_pass()

    nc.compile()
    return nc


def get_nc(nrep=None):
    if nrep not in _cache:
        _cache[nrep] = _build(nrep)
    return _cache[nrep]


def _conv_u(a32):
    """u -> device dtype, floored at the smallest nonzero code so the
    device-side Ln(u - delta) is always finite."""
    import ml_dtypes

    if U_DTYPE == "f8e3":
        q = a32.astype(ml_dtypes.float8_e3m4)
        return np.maximum(q, ml_dtypes.float8_e3m4(U8_FLOOR))
    if U_DTYPE == "f8e4":
        q = a32.astype(ml_dtypes.float8_e4m3)
        return np.maximum(q, ml_dtypes.float8_e4m3(U8E4_FLOOR))
    q = a32.astype(np.float16)
    return np.maximum(q, np.float16(2.0 ** -12))


def _conv_g(a32):
    import ml_dtypes

    if G_DTYPE == "f8e3":
        return a32.astype(ml_dtypes.float8_e3m4)
    if G_DTYPE == "f8e4":
        return a32.astype(ml_dtypes.float8_e4m3)
    return a32.astype(np.float16)


def stage_core_inputs(arr: np.ndarray, core: int, kind: str) -> np.ndarray:
    """Slice one core's [CH, 2] block and lay it out as [P, 2*FTOT]:
    within chunk i, component-0 values occupy the first Fi columns and
    component-1 the next Fi (unit-stride halves for the engines)."""
    conv = _conv_u if kind == "u" else _conv_g
    a = conv(arr[core * CH : (core + 1) * CH])
    a = a.reshape(P, FTOT, 2)
    out = np.empty((P, 2 * FTOT), a.dtype)
    for Fi, Oi in zip(SIZES, OFFS):
        blk = a[:, Oi : Oi + Fi, :]
        out[:, 2 * Oi : 2 * Oi + Fi] = blk[:, :, 0]
        out[:, 2 * Oi + Fi : 2 * (Oi + Fi)] = blk[:, :, 1]
    return out


def core_input_map(gen: np.ndarray, u: np.ndarray, core: int) -> dict:
    return {
        "gen": stage_core_inputs(gen, core, "g"),
        "u": stage_core_inputs(u, core, "u"),
    }


def run_cores(gen: np.ndarray, u: np.ndarray, trace: bool = False):
    """Run the SPMD kernel on flat [E, 2] inputs; returns (flat out [E], results obj)."""
    from concourse.bass_utils import run_bass_kernel_spmd

    nc = get_nc()
    in_maps = [core_input_map(gen, u, c) for c in range(NCORES)]
    kw = {}
    if trace:
        kw = {"trace": True, "trace_cores": list(range(NCORES)), "stitch_traces": True}
    res = run_bass_kernel_spmd(nc, in_maps, core_ids=list(range(NCORES)), **kw)
    out = np.concatenate(
        [np.asarray(r["out"]).astype(np.float32).reshape(-1) for r in res.results]
    )
    return out, res


def assemble(out: np.ndarray) -> np.ndarray:
    full = np.zeros((SZ, SZ), np.float32)
    a = out[:E1].reshape(S, DEL_NUM)
    full[:S, S:] = a
    full[S:, :S] = a.T
    ti, tj = np.triu_indices(DEL_NUM, k=1)
    b = np.zeros((DEL_NUM, DEL_NUM), np.float32)
    b[ti, tj] = out[E1:]
    full[S:, S:] = b + b.T
    return full


def kernel(gen_matrix=None, u=None, sz=None, del_num=None, **_ignored):
    gen = np.ascontiguousarray(np.asarray(gen_matrix, dtype=np.float32))
    uu = np.ascontiguousarray(np.asarray(u, dtype=np.float32))
    assert gen.shape == (E, 2) and uu.shape == (E, 2)
    out, _ = run_cores(gen, uu)
    return assemble(out)
